# revision 1
# baseline (speedup 1.0000x reference)
"""SGC graph-conv kernel for Trainium2 (8 NeuronCores, SPMD).

Computes: out = segment_sum(edge_val[:,None] * feat[edge_col], edge_row) @ W.T + b

Strategy (per sharding hint): 1D row-partition by destination. edge_row is
sorted, so each core's edges are a contiguous slice. Each core owns 12500
destination rows, processed as 100 blocks of 125 rows. Within a block, edges
are padded to a fixed multiple of 128 and processed 128 at a time:
  - indirect-DMA gather of feat rows for the 128 edge sources  -> M [128,64]
  - one fused vector op builds S[e,r] = (lrow[e]==r) * val[e]  -> S [128,125]
  - PE matmul  hT += M.T @ S  accumulates the block result in PSUM [64,125]
Then the tiny Linear is a second matmul (lhsT=hT, rhs=W.T) + bias add.

All scalar/constant data (lrow, vals, iota, W.T, bias) is packed into a single
"meta" tensor loaded by one DMA: the trn2 ISA allows only a couple of sync
waits per instruction, so consumers must not depend on many separate DMAs.
"""

import sys

sys.path.insert(0, "/opt/trn_rl_repo")

import numpy as np

N_NODES = 100000
N_EDGES = 1600000
F = 64
C = 64
N_CORES = 8
ROWS_PER_CORE = N_NODES // N_CORES  # 12500
ROWS_PER_BLOCK = 125
NB = ROWS_PER_CORE // ROWS_PER_BLOCK  # 100 blocks per core
P = 128

_CACHE = {}


def _build_program(T: int):
    from concourse import bacc, bass, mybir
    from concourse.tile import TileContext

    f32 = mybir.dt.float32
    i32 = mybir.dt.int32
    NT = NB * T
    W_META = 2 * NT + P + 2 * C  # lrow | vals | iota | wt(padded) | brep

    nc = bacc.Bacc()
    feat_d = nc.dram_tensor("feat", [N_NODES, F], f32, kind="ExternalInput")
    cols_d = nc.dram_tensor("cols", [P, NT], i32, kind="ExternalInput")
    meta_d = nc.dram_tensor("meta", [P, W_META], f32, kind="ExternalInput")
    out_d = nc.dram_tensor("out", [ROWS_PER_CORE, C], f32, kind="ExternalOutput")

    R = ROWS_PER_BLOCK
    OFF_VALS = NT
    OFF_IOTA = 2 * NT
    OFF_WT = 2 * NT + P
    OFF_B = 2 * NT + P + C

    with TileContext(nc) as tc:
        with (
            tc.tile_pool(name="edges", bufs=1) as epool,
            tc.tile_pool(name="work", bufs=6) as wpool,
            tc.tile_pool(name="psum", bufs=2, space="PSUM") as ppool,
            tc.tile_pool(name="outp", bufs=3) as opool,
        ):
            cols_sb = epool.tile([P, NT], i32)
            meta_sb = epool.tile([P, W_META], f32)
            nc.sync.dma_start(out=cols_sb[:], in_=cols_d[:])
            nc.sync.dma_start(out=meta_sb[:], in_=meta_d[:])
            iota_ap = meta_sb[:, OFF_IOTA : OFF_IOTA + R]
            wt_ap = meta_sb[:F, OFF_WT : OFF_WT + C]
            brep_ap = meta_sb[:R, OFF_B : OFF_B + C]

            for b in range(NB):
                hT_ps = ppool.tile([F, R], f32, tag="hT")
                for t in range(T):
                    k = b * T + t
                    m = wpool.tile([P, F], f32, tag="m")
                    nc.gpsimd.indirect_dma_start(
                        out=m[:],
                        out_offset=None,
                        in_=feat_d[:],
                        in_offset=bass.IndirectOffsetOnAxis(
                            ap=cols_sb[:, k : k + 1], axis=0
                        ),
                    )
                    s = wpool.tile([P, R], f32, tag="s")
                    nc.vector.tensor_scalar(
                        out=s[:],
                        in0=iota_ap,
                        scalar1=meta_sb[:, k : k + 1],
                        scalar2=meta_sb[:, OFF_VALS + k : OFF_VALS + k + 1],
                        op0=mybir.AluOpType.is_equal,
                        op1=mybir.AluOpType.mult,
                    )
                    nc.tensor.matmul(
                        out=hT_ps[:],
                        lhsT=m[:],
                        rhs=s[:],
                        start=(t == 0),
                        stop=(t == T - 1),
                    )
                hT_sb = wpool.tile([F, R], f32, tag="hTsb")
                nc.scalar.activation(
                    out=hT_sb[:], in_=hT_ps[:],
                    func=mybir.ActivationFunctionType.Copy,
                )
                o_ps = ppool.tile([R, C], f32, tag="o")
                nc.tensor.matmul(
                    out=o_ps[:], lhsT=hT_sb[:], rhs=wt_ap, start=True, stop=True
                )
                o_sb = opool.tile([R, C], f32, tag="osb")
                nc.vector.tensor_add(out=o_sb[:], in0=o_ps[:], in1=brep_ap)
                nc.sync.dma_start(
                    out=out_d[b * R : (b + 1) * R, :], in_=o_sb[:]
                )
    if not nc.is_finalized():
        nc.finalize()
    return nc


def _prep(feat, edge_row, edge_col, edge_val, W, b):
    """Host-side sharding: split sorted-by-row edge list into 800 row blocks
    (8 cores x 100 blocks x 125 rows), pad each block's edges to a common
    multiple of 128, and lay out per-tile edge data as [128, n_tiles]."""
    feat = np.ascontiguousarray(np.asarray(feat, dtype=np.float32))
    er = np.asarray(edge_row, dtype=np.int64)
    ec = np.asarray(edge_col, dtype=np.int32)
    ev = np.asarray(edge_val, dtype=np.float32)
    W = np.asarray(W, dtype=np.float32)
    b = np.asarray(b, dtype=np.float32)

    n_blocks_total = N_CORES * NB
    block_starts = np.searchsorted(
        er, np.arange(0, N_NODES + 1, ROWS_PER_BLOCK), side="left"
    )
    counts = np.diff(block_starts)
    max_cnt = int(counts.max())
    T = max(1, (max_cnt + P - 1) // P)
    BE = T * P
    NT = NB * T

    # padded per-block arrays [n_blocks_total, BE]
    cols_p = np.zeros((n_blocks_total, BE), dtype=np.int32)
    lrow_p = np.zeros((n_blocks_total, BE), dtype=np.float32)
    vals_p = np.zeros((n_blocks_total, BE), dtype=np.float32)
    lrow_all = (er % ROWS_PER_BLOCK).astype(np.float32)
    for g in range(n_blocks_total):
        s, e = block_starts[g], block_starts[g + 1]
        n = e - s
        cols_p[g, :n] = ec[s:e]
        lrow_p[g, :n] = lrow_all[s:e]
        vals_p[g, :n] = ev[s:e]

    # -> per core [128, NB*T]: tile t of block b at column b*T+t, partition=edge
    def to_core_layout(a):
        # [NB, BE] -> [NB, T, 128] -> [128, NB, T] -> [128, NB*T]
        return np.ascontiguousarray(
            a.reshape(NB, T, P).transpose(2, 0, 1).reshape(P, NB * T)
        )

    wt_pad = np.zeros((P, C), dtype=np.float32)
    wt_pad[:F, :] = W.T
    brep = np.tile(b[None, :], (P, 1)).astype(np.float32)
    iota = np.tile(np.arange(P, dtype=np.float32)[None, :], (P, 1))

    in_maps = []
    for c in range(N_CORES):
        g0, g1 = c * NB, (c + 1) * NB
        meta = np.concatenate(
            [
                to_core_layout(lrow_p[g0:g1]),
                to_core_layout(vals_p[g0:g1]),
                iota,
                wt_pad,
                brep,
            ],
            axis=1,
        )
        in_maps.append(
            {
                "feat": feat,
                "cols": to_core_layout(cols_p[g0:g1]),
                "meta": np.ascontiguousarray(meta),
            }
        )
    return T, in_maps


def kernel(feat, edge_row, edge_col, edge_val, W, b, _trace=False, _trace_kwargs=None):
    from concourse.bass_utils import run_bass_kernel_spmd

    T, in_maps = _prep(feat, edge_row, edge_col, edge_val, W, b)
    if T not in _CACHE:
        _CACHE[T] = _build_program(T)
    nc = _CACHE[T]
    kw = {}
    if _trace:
        kw["trace"] = True
        kw.update(_trace_kwargs or {})
    res = run_bass_kernel_spmd(nc, in_maps, list(range(N_CORES)), **kw)
    out = np.concatenate([r["out"] for r in res.results], axis=0)
    if _trace:
        return out, res
    return out



# revision 10
# speedup vs baseline: 1.1321x; 1.1321x over previous
"""SGC graph-conv kernel for Trainium2 (8 NeuronCores, SPMD).

Computes: out = segment_sum(edge_val[:,None] * feat[edge_col], edge_row) @ W.T + b

Strategy: 1D row-partition by destination (edge_row is sorted, so each core's
edges are a contiguous slice). Each core owns 12500 destination rows = 500
blocks of R=25 rows, processed in 20 supers of 25 blocks.

Gather: the SWDGE fixed cost (~1us) is per *call*, so per-edge indirect DMAs
are hopeless; batched gathers use InstDMAGatherAnt (dma_gather), whose int16
indices reach only 32768 rows. feat is therefore split into 4 static chunks
of 25000 rows. Each (block, chunk) bucket gets exactly one 128-slot tile
(bucket mean ~100 edges, P(>128) ~ 6e-4; the rare overflow edges spill to one
shared overflow tile per super, fetched with a classic 128-row
indirect_dma_start and routed by a super-wide one-hot). Pad slots re-fetch
chunk row 0 (valid, row-buffer hot) and are zeroed via the S matrix.

Per super: 4 dma_gather calls (3200 rows each, chunk-major into contiguous
SBUF) + 1 overflow indirect DMA -> m (f32) -> ACT copy to bf16. S one-hot
built in 4 batched DVE ops with stride-0 broadcast APs:
    eq = (lrow == iota)   s = eq * val      (pads ship lrow=1e6 -> s=0)
Each block accumulates hT[64, 25] in PSUM over its 4 chunk tiles + the
super's overflow tile (auto-masked by is_equal against iota625). Then per
5-block hT group: ACT copy to bf16, matmul vs W.T (bf16), bias add during
the PSUM->SBUF copy, one batched 625-row output DMA per super.
"""

import sys

sys.path.insert(0, "/opt/trn_rl_repo")

import numpy as np

N_NODES = 100000
N_EDGES = 1600000
F = 64
C = 64
N_CORES = 8
ROWS_PER_CORE = N_NODES // N_CORES  # 12500
P = 128
R = 25                      # rows per block
NB = ROWS_PER_CORE // R     # 500 blocks per core
NCHUNK = 4
CHUNK = N_NODES // NCHUNK   # 25000 (< 32768: int16-addressable)
SPB = 25                    # blocks per super
NSUP = NB // SPB            # 20 supers per core
TPS = NCHUNK * SPB + 1      # tiles per super (100 grid + 1 overflow)
NTILE = NSUP * TPS          # 2020 tiles per core
SROWS = SPB * R             # 625 rows per super
HGB = 5                     # blocks per hT PSUM group
IDXC = SPB * P // 16        # idx16 columns per gather call (200)

_CACHE = {}


def _build_program():
    from concourse import bacc, bass, mybir
    from concourse.ap import AP
    from concourse.tile import TileContext

    f32 = mybir.dt.float32
    bf16 = mybir.dt.bfloat16
    i32 = mybir.dt.int32
    i16 = mybir.dt.int16
    Copy = mybir.ActivationFunctionType.Copy
    Alu = mybir.AluOpType

    W_META = 2 * NTILE + R + SROWS + C + C
    OFF_LROW = 0
    OFF_VALS = NTILE
    OFF_I25 = 2 * NTILE
    OFF_I625 = 2 * NTILE + R
    OFF_WT = 2 * NTILE + R + SROWS
    OFF_B = OFF_WT + C

    GRID = NCHUNK * SPB          # 100 grid tiles per super
    SW = GRID * R + SROWS        # 3125 = s columns per super (grid + ov)

    nc = bacc.Bacc()
    feat_d = nc.dram_tensor("feat", [N_NODES, F], f32, kind="ExternalInput")
    idx_d = nc.dram_tensor("idx16", [P, NSUP * NCHUNK * IDXC], i16,
                           kind="ExternalInput")
    colsov_d = nc.dram_tensor("colsov", [P, NSUP], i32, kind="ExternalInput")
    meta_d = nc.dram_tensor("meta", [P, W_META], f32, kind="ExternalInput")
    out_d = nc.dram_tensor("out", [ROWS_PER_CORE, C], f32, kind="ExternalOutput")

    with TileContext(nc) as tc:
        with (
            tc.tile_pool(name="const", bufs=1) as cpool,
            tc.tile_pool(name="m", bufs=2) as mpool,
            tc.tile_pool(name="mb", bufs=2) as mbpool,
            tc.tile_pool(name="u", bufs=2) as upool,
            tc.tile_pool(name="s", bufs=2) as spool,
            tc.tile_pool(name="hts", bufs=3) as htspool,
            tc.tile_pool(name="osb", bufs=2) as opool,
            tc.tile_pool(name="psum_h", bufs=2, space="PSUM") as hpsum,
            tc.tile_pool(name="psum_o", bufs=2, space="PSUM") as opsum,
        ):
            idx_sb = cpool.tile([P, NSUP * NCHUNK * IDXC], i16)
            colsov_sb = cpool.tile([P, NSUP], i32)
            meta_sb = cpool.tile([P, W_META], f32)
            wt_bf = cpool.tile([F, C], bf16)
            nc.sync.dma_start(out=idx_sb[:], in_=idx_d[:])
            nc.sync.dma_start(out=colsov_sb[:], in_=colsov_d[:])
            nc.sync.dma_start(out=meta_sb[:], in_=meta_d[:])
            nc.scalar.activation(
                out=wt_bf[:], in_=meta_sb[:F, OFF_WT : OFF_WT + C], func=Copy
            )
            brep_ap = meta_sb[:125, OFF_B : OFF_B + C]

            for sp in range(NSUP):
                m = mpool.tile([P, TPS * F], f32, tag="m")
                m_ap = m[:]
                for q in range(NCHUNK):
                    out3d = AP(
                        m_ap.tensor,
                        m_ap.offset + q * SPB * F,
                        [m_ap.ap[0], [F, SPB], [1, F]],
                    )
                    cb = (sp * NCHUNK + q) * IDXC
                    nc.gpsimd.dma_gather(
                        out_ap=out3d,
                        in_ap=feat_d[q * CHUNK : (q + 1) * CHUNK],
                        idxs_ap=idx_sb[:, cb : cb + IDXC],
                        num_idxs=SPB * P,
                        num_idxs_reg=SPB * P,
                        elem_size=F,
                        single_packet=False,
                    )
                nc.gpsimd.indirect_dma_start(
                    out=m[:, GRID * F : TPS * F],
                    out_offset=None,
                    in_=feat_d[:],
                    in_offset=bass.IndirectOffsetOnAxis(
                        ap=colsov_sb[:, sp : sp + 1], axis=0
                    ),
                )
                mb = mbpool.tile([P, TPS * F], bf16, tag="mb")
                nc.scalar.activation(out=mb[:], in_=m[:], func=Copy)

                u = upool.tile([P, SW], bf16, tag="u")
                s = spool.tile([P, SW], bf16, tag="s")
                t0 = sp * TPS
                lrow_g = (
                    meta_sb[:, OFF_LROW + t0 : OFF_LROW + t0 + GRID]
                    .unsqueeze(2)
                    .broadcast_to([P, GRID, R])
                )
                vals_g = (
                    meta_sb[:, OFF_VALS + t0 : OFF_VALS + t0 + GRID]
                    .unsqueeze(2)
                    .broadcast_to([P, GRID, R])
                )
                iota_g = (
                    meta_sb[:, OFF_I25 : OFF_I25 + R]
                    .unsqueeze(1)
                    .broadcast_to([P, GRID, R])
                )
                nc.vector.scalar_tensor_tensor(
                    out=u[:, : GRID * R], in0=lrow_g, scalar=0.0, in1=iota_g,
                    op0=Alu.bypass, op1=Alu.is_equal,
                )
                nc.vector.scalar_tensor_tensor(
                    out=s[:, : GRID * R], in0=u[:, : GRID * R], scalar=0.0,
                    in1=vals_g, op0=Alu.bypass, op1=Alu.mult,
                )
                lrow_o = (
                    meta_sb[:, OFF_LROW + t0 + GRID : OFF_LROW + t0 + TPS]
                    .unsqueeze(2)
                    .broadcast_to([P, 1, SROWS])
                )
                vals_o = (
                    meta_sb[:, OFF_VALS + t0 + GRID : OFF_VALS + t0 + TPS]
                    .unsqueeze(2)
                    .broadcast_to([P, 1, SROWS])
                )
                iota_o = (
                    meta_sb[:, OFF_I625 : OFF_I625 + SROWS]
                    .unsqueeze(1)
                    .broadcast_to([P, 1, SROWS])
                )
                nc.vector.scalar_tensor_tensor(
                    out=u[:, GRID * R :], in0=lrow_o, scalar=0.0, in1=iota_o,
                    op0=Alu.bypass, op1=Alu.is_equal,
                )
                nc.vector.scalar_tensor_tensor(
                    out=s[:, GRID * R :], in0=u[:, GRID * R :], scalar=0.0,
                    in1=vals_o, op0=Alu.bypass, op1=Alu.mult,
                )

                osb = opool.tile([125, (SROWS // 125) * C], f32, tag="osb")
                for hg in range(SPB // HGB):
                    hT = hpsum.tile([F, HGB * R], f32, tag="hT")
                    for b5 in range(HGB):
                        b = hg * HGB + b5
                        for q in range(NCHUNK):
                            tl = q * SPB + b
                            nc.tensor.matmul(
                                out=hT[:, b5 * R : (b5 + 1) * R],
                                lhsT=mb[:, tl * F : (tl + 1) * F],
                                rhs=s[:, tl * R : (tl + 1) * R],
                                start=(q == 0),
                                stop=False,
                            )
                        nc.tensor.matmul(
                            out=hT[:, b5 * R : (b5 + 1) * R],
                            lhsT=mb[:, GRID * F : TPS * F],
                            rhs=s[:, GRID * R + b * R : GRID * R + (b + 1) * R],
                            start=False,
                            stop=True,
                        )
                    hts = htspool.tile([F, HGB * R], bf16, tag="hts")
                    nc.scalar.activation(out=hts[:], in_=hT[:], func=Copy)
                    o_ps = opsum.tile([125, C], f32, tag="o")
                    nc.tensor.matmul(
                        out=o_ps[:], lhsT=hts[:], rhs=wt_bf[:],
                        start=True, stop=True,
                    )
                    nc.vector.tensor_add(
                        out=osb[:, hg * C : (hg + 1) * C], in0=o_ps[:], in1=brep_ap
                    )
                ob = out_d[:]
                out_ap = AP(
                    ob.tensor,
                    sp * SROWS * C,
                    [[C, 125], [125 * C, SROWS // 125], [1, C]],
                )
                nc.sync.dma_start(out=out_ap, in_=osb[:])

    if not nc.is_finalized():
        nc.finalize()
    return nc


def _prep(feat, edge_row, edge_col, edge_val, W, b):
    """Host-side packing for the chunked-gather layout. See module docstring."""
    feat = np.ascontiguousarray(np.asarray(feat, dtype=np.float32))
    er = np.asarray(edge_row, dtype=np.int64)
    ec = np.asarray(edge_col, dtype=np.int64)
    ev = np.asarray(edge_val, dtype=np.float32)
    W = np.asarray(W, dtype=np.float32)
    b = np.asarray(b, dtype=np.float32)

    blk = er // R                       # global block 0..3999
    q = ec // CHUNK                     # chunk 0..3
    pk = blk * NCHUNK + q               # (block, chunk) bucket id
    order = np.lexsort((ec, pk))
    eb = blk[order]
    eq = q[order]
    ecol = ec[order]
    evv = ev[order]
    erow = er[order]
    pks = pk[order]

    nbuk = N_NODES // R * NCHUNK        # 16000 buckets
    counts = np.bincount(pks, minlength=nbuk)
    starts = np.concatenate([[0], np.cumsum(counts)])
    rank = np.arange(N_EDGES) - starts[pks]

    core = eb // NB
    bc = eb % NB                        # block within core
    sp = bc // SPB                      # super within core
    b_local = bc % SPB                  # block within super

    grid = rank < P
    # ---- grid placement ----
    g_core = core[grid]
    g_sp = sp[grid]
    g_q = eq[grid]
    g_bl = b_local[grid]
    g_rank = rank[grid]
    g_tile = g_sp * TPS + g_q * SPB + g_bl          # tile index within core

    NIDX = NSUP * NCHUNK * IDXC
    idx16 = np.zeros((N_CORES, 16, NIDX), dtype=np.int16)  # pad = chunk row 0
    slot = g_bl * P + g_rank
    colpos = (g_sp * NCHUNK + g_q) * IDXC + slot // 16
    idx16[g_core, slot % 16, colpos] = (ecol[grid] - eq[grid] * CHUNK).astype(
        np.int16
    )

    lrow = np.full((N_CORES, P, NTILE), 1.0e6, dtype=np.float32)
    vals = np.zeros((N_CORES, P, NTILE), dtype=np.float32)
    lrow[g_core, g_rank, g_tile] = (erow[grid] % R).astype(np.float32)
    vals[g_core, g_rank, g_tile] = evv[grid]

    # ---- overflow placement ----
    GRID = NCHUNK * SPB
    colsov = np.zeros((N_CORES, P, NSUP), dtype=np.int32)
    ovf = ~grid
    if ovf.any():
        okey = core[ovf] * NSUP + sp[ovf]
        oorder = np.argsort(okey, kind="stable")
        okey_s = okey[oorder]
        ocnt = np.bincount(okey_s, minlength=N_CORES * NSUP)
        if ocnt.max() > P:
            raise RuntimeError(f"overflow tile overflow: {ocnt.max()} > {P}")
        ostarts = np.concatenate([[0], np.cumsum(ocnt)])
        okk = np.arange(okey_s.size) - ostarts[okey_s]
        oc = core[ovf][oorder]
        osp = sp[ovf][oorder]
        o_tile = osp * TPS + GRID
        colsov[oc, okk, osp] = ecol[ovf][oorder].astype(np.int32)
        lrow[oc, okk, o_tile] = (
            erow[ovf][oorder] - (oc * ROWS_PER_CORE + osp * SROWS)
        ).astype(np.float32)
        vals[oc, okk, o_tile] = evv[ovf][oorder]

    iota25 = np.tile(np.arange(R, dtype=np.float32)[None, :], (P, 1))
    iota625 = np.tile(np.arange(SROWS, dtype=np.float32)[None, :], (P, 1))
    wt_pad = np.zeros((P, C), dtype=np.float32)
    wt_pad[:F, :] = W.T
    brep = np.tile(b[None, :], (P, 1)).astype(np.float32)

    in_maps = []
    for c in range(N_CORES):
        meta = np.concatenate(
            [lrow[c], vals[c], iota25, iota625, wt_pad, brep], axis=1
        )
        in_maps.append(
            {
                "feat": feat,
                "idx16": np.ascontiguousarray(np.tile(idx16[c], (8, 1))),
                "colsov": np.ascontiguousarray(colsov[c]),
                "meta": np.ascontiguousarray(meta),
            }
        )
    return in_maps


def kernel(feat, edge_row, edge_col, edge_val, W, b, _trace=False, _trace_kwargs=None):
    from concourse.bass_utils import run_bass_kernel_spmd

    in_maps = _prep(feat, edge_row, edge_col, edge_val, W, b)
    if "prog" not in _CACHE:
        _CACHE["prog"] = _build_program()
    nc = _CACHE["prog"]
    kw = {}
    if _trace:
        kw["trace"] = True
        kw.update(_trace_kwargs or {})
    res = run_bass_kernel_spmd(nc, in_maps, list(range(N_CORES)), **kw)
    out = np.concatenate([r["out"] for r in res.results], axis=0)
    if _trace:
        return out, res
    return out


# revision 11
# speedup vs baseline: 2.9648x; 2.6188x over previous
"""SGC graph-conv kernel for Trainium2 (8 NeuronCores, SPMD).

Computes: out = segment_sum(edge_val[:,None] * feat[edge_col], edge_row) @ W.T + b

Strategy: 1D row-partition by destination (edge_row is sorted, so each core's
edges are a contiguous slice). Each core owns 12500 destination rows = 500
blocks of R=25 rows, processed in 20 supers of 25 blocks.

Gather: the SWDGE fixed cost (~1us) is per *call*, so per-edge indirect DMAs
are hopeless; batched gathers use InstDMAGatherAnt (dma_gather), whose int16
indices reach only 32768 rows. feat is therefore split into 4 static chunks
of 25000 rows. Each (block, chunk) bucket gets exactly one 128-slot tile
(bucket mean ~100 edges, P(>128) ~ 6e-4; the rare overflow edges spill to one
shared overflow tile per super, fetched with a classic 128-row
indirect_dma_start and routed by a super-wide one-hot). Pad slots re-fetch
chunk row 0 (valid, row-buffer hot) and are zeroed via the S matrix.

Per super: 4 dma_gather calls (3200 rows each, chunk-major into contiguous
SBUF) + 1 overflow indirect DMA -> m (f32) -> ACT copy to bf16. S one-hot
built in 4 batched DVE ops with stride-0 broadcast APs:
    eq = (lrow == iota)   s = eq * val      (pads ship lrow=1e6 -> s=0)
Each block accumulates hT[64, 25] in PSUM over its 4 chunk tiles + the
super's overflow tile (auto-masked by is_equal against iota625). Then per
5-block hT group: ACT copy to bf16, matmul vs W.T (bf16), bias add during
the PSUM->SBUF copy, one batched 625-row output DMA per super.
"""

import sys

sys.path.insert(0, "/opt/trn_rl_repo")

import numpy as np

N_NODES = 100000
N_EDGES = 1600000
F = 64
C = 64
N_CORES = 8
ROWS_PER_CORE = N_NODES // N_CORES  # 12500
P = 128
R = 25                      # rows per block
NB = ROWS_PER_CORE // R     # 500 blocks per core
NCHUNK = 4
CHUNK = N_NODES // NCHUNK   # 25000 (< 32768: int16-addressable)
SPB = 25                    # blocks per super
NSUP = NB // SPB            # 20 supers per core
TPS = NCHUNK * SPB + 1      # tiles per super (100 grid + 1 overflow)
NTILE = NSUP * TPS          # 2020 tiles per core
SROWS = SPB * R             # 625 rows per super
HGB = 5                     # blocks per hT PSUM group
IDXC = SPB * P // 16        # idx16 columns per gather call (200)

_CACHE = {}


def _build_program():
    from concourse import bacc, bass, mybir
    from concourse.ap import AP
    from concourse.tile import TileContext

    f32 = mybir.dt.float32
    bf16 = mybir.dt.bfloat16
    i32 = mybir.dt.int32
    i16 = mybir.dt.int16
    Copy = mybir.ActivationFunctionType.Copy
    Alu = mybir.AluOpType

    W_META = 2 * NTILE + R + SROWS + C + C
    OFF_LROW = 0
    OFF_VALS = NTILE
    OFF_I25 = 2 * NTILE
    OFF_I625 = 2 * NTILE + R
    OFF_WT = 2 * NTILE + R + SROWS
    OFF_B = OFF_WT + C

    GRID = NCHUNK * SPB          # 100 grid tiles per super
    SW = GRID * R + SROWS        # 3125 = s columns per super (grid + ov)

    nc = bacc.Bacc(num_swdge_queues=4)
    feat_d = nc.dram_tensor("feat", [N_NODES, F], f32, kind="ExternalInput")
    idx_d = nc.dram_tensor("idx16", [P, NSUP * NCHUNK * IDXC], i16,
                           kind="ExternalInput")
    colsov_d = nc.dram_tensor("colsov", [P, NSUP], i32, kind="ExternalInput")
    meta_d = nc.dram_tensor("meta", [P, W_META], f32, kind="ExternalInput")
    out_d = nc.dram_tensor("out", [ROWS_PER_CORE, C], f32, kind="ExternalOutput")

    with TileContext(nc) as tc:
        with (
            tc.tile_pool(name="const", bufs=1) as cpool,
            tc.tile_pool(name="m", bufs=2) as mpool,
            tc.tile_pool(name="mb", bufs=2) as mbpool,
            tc.tile_pool(name="u", bufs=2) as upool,
            tc.tile_pool(name="s", bufs=2) as spool,
            tc.tile_pool(name="hts", bufs=3) as htspool,
            tc.tile_pool(name="osb", bufs=2) as opool,
            tc.tile_pool(name="psum_h", bufs=2, space="PSUM") as hpsum,
            tc.tile_pool(name="psum_o", bufs=2, space="PSUM") as opsum,
        ):
            idx_sb = cpool.tile([P, NSUP * NCHUNK * IDXC], i16)
            colsov_sb = cpool.tile([P, NSUP], i32)
            meta_sb = cpool.tile([P, W_META], f32)
            wt_bf = cpool.tile([F, C], bf16)
            nc.sync.dma_start(out=idx_sb[:], in_=idx_d[:])
            nc.sync.dma_start(out=colsov_sb[:], in_=colsov_d[:])
            nc.sync.dma_start(out=meta_sb[:], in_=meta_d[:])
            nc.scalar.activation(
                out=wt_bf[:], in_=meta_sb[:F, OFF_WT : OFF_WT + C], func=Copy
            )
            brep_ap = meta_sb[:125, OFF_B : OFF_B + C]

            for sp in range(NSUP):
                m = mpool.tile([P, TPS * F], f32, tag="m")
                m_ap = m[:]
                for q in range(NCHUNK):
                    out3d = AP(
                        m_ap.tensor,
                        m_ap.offset + q * SPB * F,
                        [m_ap.ap[0], [F, SPB], [1, F]],
                    )
                    cb = (sp * NCHUNK + q) * IDXC
                    nc.gpsimd.dma_gather(
                        out_ap=out3d,
                        in_ap=feat_d[q * CHUNK : (q + 1) * CHUNK],
                        idxs_ap=idx_sb[:, cb : cb + IDXC],
                        num_idxs=SPB * P,
                        num_idxs_reg=SPB * P,
                        elem_size=F,
                        single_packet=False,
                        queue_num=q,
                    )
                nc.gpsimd.indirect_dma_start(
                    out=m[:, GRID * F : TPS * F],
                    out_offset=None,
                    in_=feat_d[:],
                    in_offset=bass.IndirectOffsetOnAxis(
                        ap=colsov_sb[:, sp : sp + 1], axis=0
                    ),
                )
                mb = mbpool.tile([P, TPS * F], bf16, tag="mb")
                nc.scalar.activation(out=mb[:], in_=m[:], func=Copy)

                u = upool.tile([P, SW], bf16, tag="u")
                s = spool.tile([P, SW], bf16, tag="s")
                t0 = sp * TPS
                lrow_g = (
                    meta_sb[:, OFF_LROW + t0 : OFF_LROW + t0 + GRID]
                    .unsqueeze(2)
                    .broadcast_to([P, GRID, R])
                )
                vals_g = (
                    meta_sb[:, OFF_VALS + t0 : OFF_VALS + t0 + GRID]
                    .unsqueeze(2)
                    .broadcast_to([P, GRID, R])
                )
                iota_g = (
                    meta_sb[:, OFF_I25 : OFF_I25 + R]
                    .unsqueeze(1)
                    .broadcast_to([P, GRID, R])
                )
                nc.vector.scalar_tensor_tensor(
                    out=u[:, : GRID * R], in0=lrow_g, scalar=0.0, in1=iota_g,
                    op0=Alu.bypass, op1=Alu.is_equal,
                )
                nc.vector.scalar_tensor_tensor(
                    out=s[:, : GRID * R], in0=u[:, : GRID * R], scalar=0.0,
                    in1=vals_g, op0=Alu.bypass, op1=Alu.mult,
                )
                lrow_o = (
                    meta_sb[:, OFF_LROW + t0 + GRID : OFF_LROW + t0 + TPS]
                    .unsqueeze(2)
                    .broadcast_to([P, 1, SROWS])
                )
                vals_o = (
                    meta_sb[:, OFF_VALS + t0 + GRID : OFF_VALS + t0 + TPS]
                    .unsqueeze(2)
                    .broadcast_to([P, 1, SROWS])
                )
                iota_o = (
                    meta_sb[:, OFF_I625 : OFF_I625 + SROWS]
                    .unsqueeze(1)
                    .broadcast_to([P, 1, SROWS])
                )
                nc.vector.scalar_tensor_tensor(
                    out=u[:, GRID * R :], in0=lrow_o, scalar=0.0, in1=iota_o,
                    op0=Alu.bypass, op1=Alu.is_equal,
                )
                nc.vector.scalar_tensor_tensor(
                    out=s[:, GRID * R :], in0=u[:, GRID * R :], scalar=0.0,
                    in1=vals_o, op0=Alu.bypass, op1=Alu.mult,
                )

                osb = opool.tile([125, (SROWS // 125) * C], f32, tag="osb")
                for hg in range(SPB // HGB):
                    hT = hpsum.tile([F, HGB * R], f32, tag="hT")
                    for b5 in range(HGB):
                        b = hg * HGB + b5
                        for q in range(NCHUNK):
                            tl = q * SPB + b
                            nc.tensor.matmul(
                                out=hT[:, b5 * R : (b5 + 1) * R],
                                lhsT=mb[:, tl * F : (tl + 1) * F],
                                rhs=s[:, tl * R : (tl + 1) * R],
                                start=(q == 0),
                                stop=False,
                            )
                        nc.tensor.matmul(
                            out=hT[:, b5 * R : (b5 + 1) * R],
                            lhsT=mb[:, GRID * F : TPS * F],
                            rhs=s[:, GRID * R + b * R : GRID * R + (b + 1) * R],
                            start=False,
                            stop=True,
                        )
                    hts = htspool.tile([F, HGB * R], bf16, tag="hts")
                    nc.scalar.activation(out=hts[:], in_=hT[:], func=Copy)
                    o_ps = opsum.tile([125, C], f32, tag="o")
                    nc.tensor.matmul(
                        out=o_ps[:], lhsT=hts[:], rhs=wt_bf[:],
                        start=True, stop=True,
                    )
                    nc.vector.tensor_add(
                        out=osb[:, hg * C : (hg + 1) * C], in0=o_ps[:], in1=brep_ap
                    )
                ob = out_d[:]
                out_ap = AP(
                    ob.tensor,
                    sp * SROWS * C,
                    [[C, 125], [125 * C, SROWS // 125], [1, C]],
                )
                nc.sync.dma_start(out=out_ap, in_=osb[:])

    if not nc.is_finalized():
        nc.finalize()
    return nc


def _prep(feat, edge_row, edge_col, edge_val, W, b):
    """Host-side packing for the chunked-gather layout. See module docstring."""
    feat = np.ascontiguousarray(np.asarray(feat, dtype=np.float32))
    er = np.asarray(edge_row, dtype=np.int64)
    ec = np.asarray(edge_col, dtype=np.int64)
    ev = np.asarray(edge_val, dtype=np.float32)
    W = np.asarray(W, dtype=np.float32)
    b = np.asarray(b, dtype=np.float32)

    blk = er // R                       # global block 0..3999
    q = ec // CHUNK                     # chunk 0..3
    pk = blk * NCHUNK + q               # (block, chunk) bucket id
    order = np.lexsort((ec, pk))
    eb = blk[order]
    eq = q[order]
    ecol = ec[order]
    evv = ev[order]
    erow = er[order]
    pks = pk[order]

    nbuk = N_NODES // R * NCHUNK        # 16000 buckets
    counts = np.bincount(pks, minlength=nbuk)
    starts = np.concatenate([[0], np.cumsum(counts)])
    rank = np.arange(N_EDGES) - starts[pks]

    core = eb // NB
    bc = eb % NB                        # block within core
    sp = bc // SPB                      # super within core
    b_local = bc % SPB                  # block within super

    grid = rank < P
    # ---- grid placement ----
    g_core = core[grid]
    g_sp = sp[grid]
    g_q = eq[grid]
    g_bl = b_local[grid]
    g_rank = rank[grid]
    g_tile = g_sp * TPS + g_q * SPB + g_bl          # tile index within core

    NIDX = NSUP * NCHUNK * IDXC
    idx16 = np.zeros((N_CORES, 16, NIDX), dtype=np.int16)  # pad = chunk row 0
    slot = g_bl * P + g_rank
    colpos = (g_sp * NCHUNK + g_q) * IDXC + slot // 16
    idx16[g_core, slot % 16, colpos] = (ecol[grid] - eq[grid] * CHUNK).astype(
        np.int16
    )

    lrow = np.full((N_CORES, P, NTILE), 1.0e6, dtype=np.float32)
    vals = np.zeros((N_CORES, P, NTILE), dtype=np.float32)
    lrow[g_core, g_rank, g_tile] = (erow[grid] % R).astype(np.float32)
    vals[g_core, g_rank, g_tile] = evv[grid]

    # ---- overflow placement ----
    GRID = NCHUNK * SPB
    colsov = np.zeros((N_CORES, P, NSUP), dtype=np.int32)
    ovf = ~grid
    if ovf.any():
        okey = core[ovf] * NSUP + sp[ovf]
        oorder = np.argsort(okey, kind="stable")
        okey_s = okey[oorder]
        ocnt = np.bincount(okey_s, minlength=N_CORES * NSUP)
        if ocnt.max() > P:
            raise RuntimeError(f"overflow tile overflow: {ocnt.max()} > {P}")
        ostarts = np.concatenate([[0], np.cumsum(ocnt)])
        okk = np.arange(okey_s.size) - ostarts[okey_s]
        oc = core[ovf][oorder]
        osp = sp[ovf][oorder]
        o_tile = osp * TPS + GRID
        colsov[oc, okk, osp] = ecol[ovf][oorder].astype(np.int32)
        lrow[oc, okk, o_tile] = (
            erow[ovf][oorder] - (oc * ROWS_PER_CORE + osp * SROWS)
        ).astype(np.float32)
        vals[oc, okk, o_tile] = evv[ovf][oorder]

    iota25 = np.tile(np.arange(R, dtype=np.float32)[None, :], (P, 1))
    iota625 = np.tile(np.arange(SROWS, dtype=np.float32)[None, :], (P, 1))
    wt_pad = np.zeros((P, C), dtype=np.float32)
    wt_pad[:F, :] = W.T
    brep = np.tile(b[None, :], (P, 1)).astype(np.float32)

    in_maps = []
    for c in range(N_CORES):
        meta = np.concatenate(
            [lrow[c], vals[c], iota25, iota625, wt_pad, brep], axis=1
        )
        in_maps.append(
            {
                "feat": feat,
                "idx16": np.ascontiguousarray(np.tile(idx16[c], (8, 1))),
                "colsov": np.ascontiguousarray(colsov[c]),
                "meta": np.ascontiguousarray(meta),
            }
        )
    return in_maps


def kernel(feat, edge_row, edge_col, edge_val, W, b, _trace=False, _trace_kwargs=None):
    from concourse.bass_utils import run_bass_kernel_spmd

    in_maps = _prep(feat, edge_row, edge_col, edge_val, W, b)
    if "prog" not in _CACHE:
        _CACHE["prog"] = _build_program()
    nc = _CACHE["prog"]
    kw = {}
    if _trace:
        kw["trace"] = True
        kw.update(_trace_kwargs or {})
    res = run_bass_kernel_spmd(nc, in_maps, list(range(N_CORES)), **kw)
    out = np.concatenate([r["out"] for r in res.results], axis=0)
    if _trace:
        return out, res
    return out


# revision 12
# speedup vs baseline: 3.2952x; 1.1114x over previous
"""SGC graph-conv kernel for Trainium2 (8 NeuronCores, SPMD).

Computes: out = segment_sum(edge_val[:,None] * feat[edge_col], edge_row) @ W.T + b

Strategy: 1D row-partition by destination (edge_row is sorted, so each core's
edges are a contiguous slice). Each core owns 12500 destination rows, padded
to 12544 = 448 blocks of R=28 rows, processed in 16 supers of 28 blocks.

Gather: indexed DMA descriptors cost ~3.4ns each (SWDGE gen + queue drain,
4 parallel SWDGE queues), so descriptor COUNT is the wall. Batched gathers
use InstDMAGatherAnt (dma_gather), whose int16 indices reach only 32768 rows;
feat is split into 4 static chunks of 25000 rows, one gather call per
(super, chunk) on its own SWDGE queue. Each (block, chunk) bucket gets
exactly one 128-slot tile (bucket mean ~112 edges = 87.5% fill; overflow
edges spill to one shared overflow tile per super, fetched with a classic
128-row indirect_dma_start and routed by a super-wide one-hot). Pad slots
re-fetch chunk row 0 (valid, row-buffer hot) and are zeroed via S.

Per super: 4 dma_gather calls (3584 rows each, chunk-major into contiguous
SBUF) + 1 overflow indirect DMA -> m (f32) -> ACT copy to bf16. S one-hot
built in 4 batched DVE ops with stride-0 broadcast APs:
    eq = (lrow == iota)   s = eq * val      (pads ship lrow=1e6 -> s=0)
Each block accumulates hT[64, 28] in PSUM over its 4 chunk tiles + the
super's overflow tile (auto-masked by is_equal against iota784). Then per
4-block hT group: ACT copy to bf16, matmul vs W.T (bf16), bias add during
the PSUM->SBUF copy, one batched 784-row output DMA per super. The padded
output rows (12500..12543) are dropped on the host.
"""

import sys

sys.path.insert(0, "/opt/trn_rl_repo")

import numpy as np

N_NODES = 100000
N_EDGES = 1600000
F = 64
C = 64
N_CORES = 8
ROWS_PER_CORE = N_NODES // N_CORES  # 12500
P = 128
R = 28                      # rows per block
RPC_PAD = 12544             # padded rows per core (= 448 * 28)
NB = RPC_PAD // R           # 448 blocks per core
NCHUNK = 4
CHUNK = N_NODES // NCHUNK   # 25000 (< 32768: int16-addressable)
SPB = 28                    # blocks per super
NSUP = NB // SPB            # 16 supers per core
TPS = NCHUNK * SPB + 1      # tiles per super (112 grid + 1 overflow)
NTILE = NSUP * TPS          # 1808 tiles per core
SROWS = SPB * R             # 784 rows per super
HGB = 4                     # blocks per hT PSUM group
IDXC = SPB * P // 16        # idx16 columns per gather call (224)

_CACHE = {}


def _build_program():
    from concourse import bacc, bass, mybir
    from concourse.ap import AP
    from concourse.tile import TileContext

    f32 = mybir.dt.float32
    bf16 = mybir.dt.bfloat16
    i32 = mybir.dt.int32
    i16 = mybir.dt.int16
    Copy = mybir.ActivationFunctionType.Copy
    Alu = mybir.AluOpType

    W_META = 2 * NTILE + R + SROWS + C + C
    OFF_LROW = 0
    OFF_VALS = NTILE
    OFF_I28 = 2 * NTILE
    OFF_I784 = 2 * NTILE + R
    OFF_WT = 2 * NTILE + R + SROWS
    OFF_B = OFF_WT + C

    GRID = NCHUNK * SPB          # 112 grid tiles per super
    SW = GRID * R + SROWS        # s columns per super (grid + ov)
    NHG = SPB // HGB             # 7 hT groups per super
    HR = HGB * R                 # 112 rows per hT group

    nc = bacc.Bacc(num_swdge_queues=4)
    feat_d = nc.dram_tensor("feat", [N_NODES, F], f32, kind="ExternalInput")
    idx_d = nc.dram_tensor("idx16", [P, NSUP * NCHUNK * IDXC], i16,
                           kind="ExternalInput")
    colsov_d = nc.dram_tensor("colsov", [P, NSUP], i32, kind="ExternalInput")
    meta_d = nc.dram_tensor("meta", [P, W_META], f32, kind="ExternalInput")
    out_d = nc.dram_tensor("out", [RPC_PAD, C], f32, kind="ExternalOutput")

    with TileContext(nc) as tc:
        with (
            tc.tile_pool(name="const", bufs=1) as cpool,
            tc.tile_pool(name="m", bufs=2) as mpool,
            tc.tile_pool(name="mb", bufs=2) as mbpool,
            tc.tile_pool(name="u", bufs=2) as upool,
            tc.tile_pool(name="s", bufs=2) as spool,
            tc.tile_pool(name="hts", bufs=3) as htspool,
            tc.tile_pool(name="osb", bufs=2) as opool,
            tc.tile_pool(name="psum_h", bufs=2, space="PSUM") as hpsum,
            tc.tile_pool(name="psum_o", bufs=2, space="PSUM") as opsum,
        ):
            idx_sb = cpool.tile([P, NSUP * NCHUNK * IDXC], i16)
            colsov_sb = cpool.tile([P, NSUP], i32)
            meta_sb = cpool.tile([P, W_META], f32)
            wt_bf = cpool.tile([F, C], bf16)
            nc.sync.dma_start(out=idx_sb[:], in_=idx_d[:])
            nc.sync.dma_start(out=colsov_sb[:], in_=colsov_d[:])
            nc.sync.dma_start(out=meta_sb[:], in_=meta_d[:])
            nc.scalar.activation(
                out=wt_bf[:], in_=meta_sb[:F, OFF_WT : OFF_WT + C], func=Copy
            )
            brep_ap = meta_sb[:HR, OFF_B : OFF_B + C]

            for sp in range(NSUP):
                m = mpool.tile([P, TPS * F], f32, tag="m")
                m_ap = m[:]
                for q in range(NCHUNK):
                    out3d = AP(
                        m_ap.tensor,
                        m_ap.offset + q * SPB * F,
                        [m_ap.ap[0], [F, SPB], [1, F]],
                    )
                    cb = (sp * NCHUNK + q) * IDXC
                    nc.gpsimd.dma_gather(
                        out_ap=out3d,
                        in_ap=feat_d[q * CHUNK : (q + 1) * CHUNK],
                        idxs_ap=idx_sb[:, cb : cb + IDXC],
                        num_idxs=SPB * P,
                        num_idxs_reg=SPB * P,
                        elem_size=F,
                        single_packet=False,
                        queue_num=q,
                    )
                nc.gpsimd.indirect_dma_start(
                    out=m[:, GRID * F : TPS * F],
                    out_offset=None,
                    in_=feat_d[:],
                    in_offset=bass.IndirectOffsetOnAxis(
                        ap=colsov_sb[:, sp : sp + 1], axis=0
                    ),
                )
                mb = mbpool.tile([P, TPS * F], bf16, tag="mb")
                nc.scalar.activation(out=mb[:], in_=m[:], func=Copy)

                u = upool.tile([P, SW], bf16, tag="u")
                s = spool.tile([P, SW], bf16, tag="s")
                t0 = sp * TPS
                lrow_g = (
                    meta_sb[:, OFF_LROW + t0 : OFF_LROW + t0 + GRID]
                    .unsqueeze(2)
                    .broadcast_to([P, GRID, R])
                )
                vals_g = (
                    meta_sb[:, OFF_VALS + t0 : OFF_VALS + t0 + GRID]
                    .unsqueeze(2)
                    .broadcast_to([P, GRID, R])
                )
                iota_g = (
                    meta_sb[:, OFF_I28 : OFF_I28 + R]
                    .unsqueeze(1)
                    .broadcast_to([P, GRID, R])
                )
                nc.vector.scalar_tensor_tensor(
                    out=u[:, : GRID * R], in0=lrow_g, scalar=0.0, in1=iota_g,
                    op0=Alu.bypass, op1=Alu.is_equal,
                )
                nc.vector.scalar_tensor_tensor(
                    out=s[:, : GRID * R], in0=u[:, : GRID * R], scalar=0.0,
                    in1=vals_g, op0=Alu.bypass, op1=Alu.mult,
                )
                lrow_o = (
                    meta_sb[:, OFF_LROW + t0 + GRID : OFF_LROW + t0 + TPS]
                    .unsqueeze(2)
                    .broadcast_to([P, 1, SROWS])
                )
                vals_o = (
                    meta_sb[:, OFF_VALS + t0 + GRID : OFF_VALS + t0 + TPS]
                    .unsqueeze(2)
                    .broadcast_to([P, 1, SROWS])
                )
                iota_o = (
                    meta_sb[:, OFF_I784 : OFF_I784 + SROWS]
                    .unsqueeze(1)
                    .broadcast_to([P, 1, SROWS])
                )
                nc.vector.scalar_tensor_tensor(
                    out=u[:, GRID * R :], in0=lrow_o, scalar=0.0, in1=iota_o,
                    op0=Alu.bypass, op1=Alu.is_equal,
                )
                nc.vector.scalar_tensor_tensor(
                    out=s[:, GRID * R :], in0=u[:, GRID * R :], scalar=0.0,
                    in1=vals_o, op0=Alu.bypass, op1=Alu.mult,
                )

                osb = opool.tile([HR, NHG * C], f32, tag="osb")
                for hg in range(NHG):
                    hT = hpsum.tile([F, HR], f32, tag="hT")
                    for b4 in range(HGB):
                        b = hg * HGB + b4
                        for q in range(NCHUNK):
                            tl = q * SPB + b
                            nc.tensor.matmul(
                                out=hT[:, b4 * R : (b4 + 1) * R],
                                lhsT=mb[:, tl * F : (tl + 1) * F],
                                rhs=s[:, tl * R : (tl + 1) * R],
                                start=(q == 0),
                                stop=False,
                            )
                        nc.tensor.matmul(
                            out=hT[:, b4 * R : (b4 + 1) * R],
                            lhsT=mb[:, GRID * F : TPS * F],
                            rhs=s[:, GRID * R + b * R : GRID * R + (b + 1) * R],
                            start=False,
                            stop=True,
                        )
                    hts = htspool.tile([F, HR], bf16, tag="hts")
                    nc.scalar.activation(out=hts[:], in_=hT[:], func=Copy)
                    o_ps = opsum.tile([HR, C], f32, tag="o")
                    nc.tensor.matmul(
                        out=o_ps[:], lhsT=hts[:], rhs=wt_bf[:],
                        start=True, stop=True,
                    )
                    nc.vector.tensor_add(
                        out=osb[:, hg * C : (hg + 1) * C], in0=o_ps[:], in1=brep_ap
                    )
                ob = out_d[:]
                out_ap = AP(
                    ob.tensor,
                    sp * SROWS * C,
                    [[C, HR], [HR * C, NHG], [1, C]],
                )
                nc.sync.dma_start(out=out_ap, in_=osb[:])

    if not nc.is_finalized():
        nc.finalize()
    return nc


def _prep(feat, edge_row, edge_col, edge_val, W, b):
    """Host-side packing for the chunked-gather layout. See module docstring."""
    feat = np.ascontiguousarray(np.asarray(feat, dtype=np.float32))
    er = np.asarray(edge_row, dtype=np.int64)
    ec = np.asarray(edge_col, dtype=np.int64)
    ev = np.asarray(edge_val, dtype=np.float32)
    W = np.asarray(W, dtype=np.float32)
    b = np.asarray(b, dtype=np.float32)

    core = er // ROWS_PER_CORE          # 0..7
    lr = er % ROWS_PER_CORE             # core-local row
    blk = lr // R                       # block within core 0..447
    q = ec // CHUNK                     # chunk 0..3
    pk = (core * NB + blk) * NCHUNK + q
    order = np.lexsort((ec, pk))
    ecol = ec[order]
    evv = ev[order]
    pks = pk[order]
    lrs = lr[order]
    eqs = q[order]

    nbuk = N_CORES * NB * NCHUNK
    counts = np.bincount(pks, minlength=nbuk)
    starts = np.concatenate([[0], np.cumsum(counts)])
    rank = np.arange(N_EDGES) - starts[pks]

    cores = pks // (NB * NCHUNK)
    blks = (pks // NCHUNK) % NB
    sps = blks // SPB
    bls = blks % SPB

    grid = rank < P
    g_core = cores[grid]
    g_sp = sps[grid]
    g_q = eqs[grid]
    g_bl = bls[grid]
    g_rank = rank[grid]
    g_tile = g_sp * TPS + g_q * SPB + g_bl

    NIDX = NSUP * NCHUNK * IDXC
    idx16 = np.zeros((N_CORES, 16, NIDX), dtype=np.int16)  # pad = chunk row 0
    slot = g_bl * P + g_rank
    colpos = (g_sp * NCHUNK + g_q) * IDXC + slot // 16
    idx16[g_core, slot % 16, colpos] = (ecol[grid] - eqs[grid] * CHUNK).astype(
        np.int16
    )

    lrow = np.full((N_CORES, P, NTILE), 1.0e6, dtype=np.float32)
    vals = np.zeros((N_CORES, P, NTILE), dtype=np.float32)
    lrow[g_core, g_rank, g_tile] = (lrs[grid] % R).astype(np.float32)
    vals[g_core, g_rank, g_tile] = evv[grid]

    GRID = NCHUNK * SPB
    colsov = np.zeros((N_CORES, P, NSUP), dtype=np.int32)
    ovf = ~grid
    if ovf.any():
        okey = cores[ovf] * NSUP + sps[ovf]
        oorder = np.argsort(okey, kind="stable")
        okey_s = okey[oorder]
        ocnt = np.bincount(okey_s, minlength=N_CORES * NSUP)
        if ocnt.max() > P:
            raise RuntimeError(f"overflow tile overflow: {ocnt.max()} > {P}")
        ostarts = np.concatenate([[0], np.cumsum(ocnt)])
        okk = np.arange(okey_s.size) - ostarts[okey_s]
        oc = cores[ovf][oorder]
        osp = sps[ovf][oorder]
        o_tile = osp * TPS + GRID
        colsov[oc, okk, osp] = ecol[ovf][oorder].astype(np.int32)
        lrow[oc, okk, o_tile] = (lrs[ovf][oorder] - osp * SROWS).astype(
            np.float32
        )
        vals[oc, okk, o_tile] = evv[ovf][oorder]

    iota28 = np.tile(np.arange(R, dtype=np.float32)[None, :], (P, 1))
    iota784 = np.tile(np.arange(SROWS, dtype=np.float32)[None, :], (P, 1))
    wt_pad = np.zeros((P, C), dtype=np.float32)
    wt_pad[:F, :] = W.T
    brep = np.tile(b[None, :], (P, 1)).astype(np.float32)

    in_maps = []
    for c in range(N_CORES):
        meta = np.concatenate(
            [lrow[c], vals[c], iota28, iota784, wt_pad, brep], axis=1
        )
        in_maps.append(
            {
                "feat": feat,
                "idx16": np.ascontiguousarray(np.tile(idx16[c], (8, 1))),
                "colsov": np.ascontiguousarray(colsov[c]),
                "meta": np.ascontiguousarray(meta),
            }
        )
    return in_maps


def kernel(feat, edge_row, edge_col, edge_val, W, b, _trace=False, _trace_kwargs=None):
    from concourse.bass_utils import run_bass_kernel_spmd

    in_maps = _prep(feat, edge_row, edge_col, edge_val, W, b)
    if "prog" not in _CACHE:
        _CACHE["prog"] = _build_program()
    nc = _CACHE["prog"]
    kw = {}
    if _trace:
        kw["trace"] = True
        kw.update(_trace_kwargs or {})
    res = run_bass_kernel_spmd(nc, in_maps, list(range(N_CORES)), **kw)
    out = np.concatenate([r["out"][:ROWS_PER_CORE] for r in res.results], axis=0)
    if _trace:
        return out, res
    return out
